# revision 1
# baseline (speedup 1.0000x reference)
"""VQ codebook decoder on 8 Trainium2 NeuronCores.

Strategy: data-parallel over tokens, but tokens are assigned to cores in
globally index-sorted order. Each core then deduplicates its 2304 tokens to
at most U_PAD=1024 unique codebook entries (seed-independent bound checked at
runtime, with a dense fallback):

  Phase A: indirect-DMA gather the core's unique codebook rows (bf16,
    transposed layout straight from the gather), run the 2-layer MLP
    (1024 -> 4096 gelu -> 1024) in bf16 on the tensor engine with fp32
    PSUM accumulation, write decoded rows to a DRAM table.
  Phase B: per 128-token block, indirect-DMA gather decoded rows by each
    token's dedup rank (f32) and DMA to the output. Blocks only read the
    prefix of the decoded table they need (ranks of sorted tokens are
    monotone), so Phase B overlaps Phase A.

The host applies the inverse token permutation when unsharding. mm1
produces h transposed ([H-part, tok]) so mm2 contracts over H without any
on-chip transpose; gelu+bias ride the scalar-engine PSUM eviction, the
output bias rides the vector-engine eviction.
"""

import sys

if "/opt/trn_rl_repo" not in sys.path:
    sys.path.insert(0, "/opt/trn_rl_repo")

import numpy as np
import ml_dtypes

import concourse.bass as bass
import concourse.mybir as mybir
import concourse.tile as tile
from concourse import bacc
from concourse.bass_utils import run_bass_kernel_spmd

B, M = 32, 576
CB, D, H, O = 8192, 1024, 4096, 1024
N_CORES = 8
T_TOTAL = B * M          # 18432
T = T_TOTAL // N_CORES   # 2304 tokens per core
P = 128
DK = D // P              # 8  k-subtiles for mm1
HK = H // P              # 32 k-subtiles for mm2
NO = O // 512            # 2  output column halves

U_PAD = 1024             # max unique codebook rows per core (dedup path)
US = 256                 # unique-slice size (Phase A granularity)
TB = 128                 # token block size (Phase B granularity)
NTB = T // TB            # 18 token blocks

BF16 = mybir.dt.bfloat16
F32 = mybir.dt.float32

_cache: dict = {}


def _wrap16(v):
    """int16 index layout for dma_gather: token j at [j%16, j//16], the
    16-row block replicated 8x down the 128 partitions."""
    v = np.asarray(v).astype(np.int16)
    return np.ascontiguousarray(np.tile(v.reshape(-1, 16).T, (8, 1)))


def _mlp_slice(nc, w1sb, w2sb, b1sb, b2sb, qpool, hpool, opool, p1pool,
               p2pool, cb16, idxsb, idx_col0, n_tok, store):
    """Gather n_tok codebook rows (by idx cols starting at idx_col0) and run
    the MLP; store(t2, osb) consumes each 128-row fp32 output block."""
    qT = qpool.tile([P, DK, n_tok], BF16, name="qT")
    nc.gpsimd.dma_gather(
        qT[:, :, :], cb16[:, :],
        idxsb[:, idx_col0:idx_col0 + n_tok // 16],
        n_tok, n_tok, D, transpose=True,
    )
    hT = hpool.tile([P, HK, n_tok], BF16, name="hT")
    for h in range(HK):
        ps1 = p1pool.tile([P, n_tok], F32, name="ps1")
        for ks in range(DK):
            nc.tensor.matmul(
                ps1[:, :],
                w1sb[:, ks, h * P:(h + 1) * P],
                qT[:, ks, :],
                start=(ks == 0), stop=(ks == DK - 1),
            )
        nc.scalar.activation(
            hT[:, h, :], ps1[:, :],
            mybir.ActivationFunctionType.Gelu_apprx_tanh,
            bias=b1sb[:, h:h + 1],
        )
    for t2 in range(n_tok // P):
        osb = opool.tile([P, O], F32, name="osb")
        for o in range(NO):
            ps2 = p2pool.tile([P, 512], F32, name="ps2")
            for ks in range(HK):
                nc.tensor.matmul(
                    ps2[:, :],
                    hT[:, ks, t2 * P:(t2 + 1) * P],
                    w2sb[:, ks, o * 512:(o + 1) * 512],
                    start=(ks == 0), stop=(ks == HK - 1),
                )
            nc.vector.tensor_add(
                osb[:, o * 512:(o + 1) * 512], ps2[:, :],
                b2sb[:, o * 512:(o + 1) * 512],
            )
        store(t2, osb)


def _declare_common(nc):
    cb16 = nc.declare_dram_parameter("cb16", [CB, D], BF16, isOutput=False)
    w1 = nc.declare_dram_parameter("w1", [D, H], BF16, isOutput=False)
    w2 = nc.declare_dram_parameter("w2", [H, O], BF16, isOutput=False)
    b1r = nc.declare_dram_parameter("b1r", [P, HK], F32, isOutput=False)
    b2r = nc.declare_dram_parameter("b2r", [P, O], F32, isOutput=False)
    return cb16, w1, w2, b1r, b2r


def _load_weights(nc, wpool, w1, w2, b1r, b2r):
    b1sb = wpool.tile([P, HK], F32)
    nc.sync.dma_start(out=b1sb[:], in_=b1r[:])
    # w1 split by h-column ranges to match mm1's consumption order (h-major),
    # so the first matmuls can start as soon as the first chunk lands.
    # w1sb[p, ks, h] = W1[ks*128+p, h]
    w1sb = wpool.tile([P, DK, H], BF16)
    w1v = w1.rearrange("(ks p) h -> p ks h", p=P)
    HC = 512
    for h0 in range(0, H, HC):
        nc.sync.dma_start(out=w1sb[:, :, h0:h0 + HC], in_=w1v[:, :, h0:h0 + HC])
    b2sb = wpool.tile([P, O], F32)
    nc.sync.dma_start(out=b2sb[:], in_=b2r[:])
    w2sb = wpool.tile([P, HK, O], BF16)
    w2v = w2.rearrange("(ks p) o -> p ks o", p=P)
    for ks in range(0, HK, 8):
        nc.sync.dma_start(out=w2sb[:, ks:ks + 8, :], in_=w2v[:, ks:ks + 8, :])
    return w1sb, w2sb, b1sb, b2sb


def _build_dedup(schedule, repeats: int = 1, us: int = US):
    """schedule[i] = number of 128-row dec blocks token block i needs."""
    NU = U_PAD // us
    nc = bacc.Bacc("TRN2", target_bir_lowering=False, debug=False,
                   num_devices=N_CORES)
    cb16, w1, w2, b1r, b2r = _declare_common(nc)
    uidx16 = nc.declare_dram_parameter("uidx16", [P, U_PAD // 16],
                                       mybir.dt.int16, isOutput=False)
    rank16 = nc.declare_dram_parameter("rank16", [P, T // 16],
                                       mybir.dt.int16, isOutput=False)
    out = nc.declare_dram_parameter("out", [T, O], F32, isOutput=True)

    with tile.TileContext(nc) as tc:
        with (
            tc.tile_pool(name="wpool", bufs=1) as wpool,
            tc.tile_pool(name="qpool", bufs=2) as qpool,
            tc.tile_pool(name="hpool", bufs=2 if us <= 256 else 1) as hpool,
            tc.tile_pool(name="opool", bufs=2) as opool,
            tc.tile_pool(name="g2pool", bufs=2) as g2pool,
            tc.tile_pool(name="dpool", bufs=1, space="DRAM") as dpool,
            tc.tile_pool(name="p1pool", bufs=4, space="PSUM") as p1pool,
            tc.tile_pool(name="p2pool", bufs=2, space="PSUM") as p2pool,
        ):
          for _rep in range(repeats):
            uidxsb = wpool.tile([P, U_PAD // 16], mybir.dt.int16)
            nc.sync.dma_start(out=uidxsb[:], in_=uidx16[:])
            w1sb, w2sb, b1sb, b2sb = _load_weights(nc, wpool, w1, w2, b1r, b2r)
            ranksb = wpool.tile([P, T // 16], mybir.dt.int16)
            nc.sync.dma_start(out=ranksb[:], in_=rank16[:])
            dec = dpool.tile([U_PAD, O], F32)

            def emit_tok_block(i):
                # schedule[i] counts 128-row dec blocks this token block needs
                need = P * schedule[i]
                g2 = g2pool.tile([P, 1, O], F32, name="g2")
                nc.gpsimd.dma_gather(
                    g2[:, :, :], dec[0:need, :],
                    ranksb[:, i * (TB // 16):(i + 1) * (TB // 16)],
                    TB, TB, O,
                )
                nc.sync.dma_start(out=out[i * TB:(i + 1) * TB, :],
                                  in_=g2[:, 0, :])

            state = {"emitted": 0, "dec_done": 0}

            def after_store():
                state["dec_done"] += 1
                while (state["emitted"] < NTB
                       and schedule[state["emitted"]] <= state["dec_done"]):
                    emit_tok_block(state["emitted"])
                    state["emitted"] += 1

            for j in range(NU):
                def store(t2, osb, j=j):
                    row = j * us + t2 * P
                    nc.sync.dma_start(out=dec[row:row + P, :], in_=osb[:])
                    after_store()
                _mlp_slice(nc, w1sb, w2sb, b1sb, b2sb, qpool, hpool, opool,
                           p1pool, p2pool, cb16, uidxsb, j * (us // 16), us,
                           store)
            while state["emitted"] < NTB:
                emit_tok_block(state["emitted"])
                state["emitted"] += 1

    nc.compile()
    return nc


def _build_dense(repeats: int = 1):
    """Fallback: straight data-parallel, no dedup (2304 tokens per core)."""
    nc = bacc.Bacc("TRN2", target_bir_lowering=False, debug=False,
                   num_devices=N_CORES)
    cb16, w1, w2, b1r, b2r = _declare_common(nc)
    idx16 = nc.declare_dram_parameter("idx16", [P, T // 16], mybir.dt.int16,
                                      isOutput=False)
    out = nc.declare_dram_parameter("out", [T, O], F32, isOutput=True)
    TS = 256

    with tile.TileContext(nc) as tc:
        with (
            tc.tile_pool(name="wpool", bufs=1) as wpool,
            tc.tile_pool(name="qpool", bufs=2) as qpool,
            tc.tile_pool(name="hpool", bufs=2) as hpool,
            tc.tile_pool(name="opool", bufs=3) as opool,
            tc.tile_pool(name="p1pool", bufs=4, space="PSUM") as p1pool,
            tc.tile_pool(name="p2pool", bufs=2, space="PSUM") as p2pool,
        ):
          for _rep in range(repeats):
            idxsb = wpool.tile([P, T // 16], mybir.dt.int16)
            nc.sync.dma_start(out=idxsb[:], in_=idx16[:])
            w1sb, w2sb, b1sb, b2sb = _load_weights(nc, wpool, w1, w2, b1r, b2r)
            for i in range(T // TS):
                def store(t2, osb, i=i):
                    row = i * TS + t2 * P
                    nc.sync.dma_start(out=out[row:row + P, :], in_=osb[:])
                _mlp_slice(nc, w1sb, w2sb, b1sb, b2sb, qpool, hpool, opool,
                           p1pool, p2pool, cb16, idxsb, i * (TS // 16), TS,
                           store)

    nc.compile()
    return nc


def _get_nc(kind, schedule=None, repeats=1, us=US):
    key = (kind, schedule, repeats, us)
    if key not in _cache:
        if kind == "dedup":
            _cache[key] = _build_dedup(schedule, repeats, us)
        else:
            _cache[key] = _build_dense(repeats)
    return _cache[key]


def _prep_weights(codebook, W1, b1, W2, b2):
    bf = ml_dtypes.bfloat16
    return {
        "cb16": np.ascontiguousarray(codebook.astype(bf)),
        "w1": np.ascontiguousarray(W1.astype(bf)),
        "w2": np.ascontiguousarray(W2.astype(bf)),
        "b1r": np.ascontiguousarray(b1.astype(np.float32).reshape(HK, P).T),
        "b2r": np.ascontiguousarray(
            np.broadcast_to(b2.astype(np.float32)[None, :], (P, O))),
    }


def _plan_dedup(index):
    """Sorted-index sharding + per-core dedup. Returns None if any core
    exceeds U_PAD unique rows (caller falls back to the dense kernel)."""
    idx_flat = np.asarray(index).reshape(-1)
    order = np.argsort(idx_flat, kind="stable")
    perms, uidxs, ranks, needs = [], [], [], []
    for c in range(N_CORES):
        perm = order[c * T:(c + 1) * T]
        vals = idx_flat[perm]
        uniq, inv = np.unique(vals, return_inverse=True)
        if uniq.size > U_PAD:
            return None
        up = np.zeros(U_PAD, np.int64)
        up[:uniq.size] = uniq
        perms.append(perm)
        uidxs.append(_wrap16(up))
        ranks.append(_wrap16(inv))
        # 128-row dec block count needed by each 128-token block of this core
        need = [int(np.ceil((inv[i * TB:(i + 1) * TB].max() + 1) / P))
                for i in range(NTB)]
        needs.append(need)
    schedule = tuple(max(needs[c][i] for c in range(N_CORES))
                     for i in range(NTB))
    return perms, uidxs, ranks, schedule


def kernel(index, codebook, W1, b1, W2, b2):
    wmaps = _prep_weights(codebook, W1, b1, W2, b2)
    plan = _plan_dedup(index)
    if plan is not None:
        perms, uidxs, ranks, schedule = plan
        nc = _get_nc("dedup", schedule)
        in_maps = [{**wmaps, "uidx16": uidxs[c], "rank16": ranks[c]}
                   for c in range(N_CORES)]
        res = run_bass_kernel_spmd(nc, in_maps, list(range(N_CORES)))
        out = np.empty((T_TOTAL, O), np.float32)
        for c in range(N_CORES):
            out[perms[c]] = res.results[c]["out"]
    else:
        nc = _get_nc("dense")
        idx_flat = np.asarray(index).reshape(-1)
        in_maps = [{**wmaps, "idx16": _wrap16(idx_flat[c * T:(c + 1) * T])}
                   for c in range(N_CORES)]
        res = run_bass_kernel_spmd(nc, in_maps, list(range(N_CORES)))
        out = np.concatenate([res.results[c]["out"] for c in range(N_CORES)],
                             axis=0)
    return out.reshape(B, M, O).astype(np.float32)



# revision 3
# speedup vs baseline: 2.9978x; 2.9978x over previous
"""VQ codebook decoder on 8 Trainium2 NeuronCores.

Strategy: the decoder output depends only on which codebook row each token
selects, so decode each *unique* referenced codebook row exactly once and
look tokens up afterwards. The host computes the global unique index set
(np.unique), pre-gathers those rows from the codebook (bf16, already in the
transposed layout mm1 wants), and splits them evenly across the 8 cores
(~U/8 rows each, padded to a multiple of 32). Each core runs the dense
2-layer MLP (1024 -> 4096 gelu -> 1024) over its rows in bf16 with fp32
PSUM accumulation and writes a decoded table; the host gathers per-token
rows from the concatenated tables during unshard (replacing the inverse
permutation scatter a per-token device output would need anyway).

Both matmuls keep weights stationary and stream tokens as the moving
operand, so tensor-engine time scales with the exact row count instead of
rounding up to 128-token tiles. Tokens are processed in two PSUM groups
(<=512 each) per weight block, issued back-to-back so each stationary
weight load is reused. Weights stream through small SBUF rings on the
Activation-engine HWDGE queue while the q rows load k-interleaved across
both queues, so the first matmul issues ~2us in. gelu+b1 ride the
scalar-engine PSUM eviction; the b2 eviction of the two final PSUM groups
runs on vector and scalar in parallel to shorten the tail.
"""

import sys

if "/opt/trn_rl_repo" not in sys.path:
    sys.path.insert(0, "/opt/trn_rl_repo")

import numpy as np
import ml_dtypes

import concourse.bass as bass
import concourse.mybir as mybir
import concourse.tile as tile
from concourse import bacc
from concourse.bass_utils import run_bass_kernel_spmd

B, M = 32, 576
CB, D, H, O = 8192, 1024, 4096, 1024
N_CORES = 8
P = 128
DK = D // P   # 8   k-subtiles for mm1
HB = H // P   # 32  column blocks of W1 / k-subtiles for mm2
OB = O // P   # 8   column blocks of W2

BF16 = mybir.dt.bfloat16
F32 = mybir.dt.float32
GELU = mybir.ActivationFunctionType.Gelu_apprx_tanh
IDENT = mybir.ActivationFunctionType.Identity

_cache: dict = {}


def _build(uc: int, repeats: int = 1):
    """MLP over `uc` codebook rows: dec[ob*128+p, t] = rec[t, ob*128+p]."""
    g1 = (uc + 31) // 32 * 16  # balanced halves, 16-aligned
    g1 = min(g1, uc)
    g2 = uc - g1
    assert 0 < uc <= 1024 and uc % 32 == 0 and g1 <= 512 and g2 <= 512

    nc = bacc.Bacc("TRN2", target_bir_lowering=False, debug=False,
                   num_devices=N_CORES)
    # qt[p, k, t] = q[t, k*128+p]; w1h[p, hb, k, c] = W1[k*128+p, hb*128+c]
    # w2h[p, ob, k, c] = W2[k*128+p, ob*128+c]; b1h/b2h column-blocked.
    qt = nc.declare_dram_parameter("qt", [P, DK, uc], BF16, isOutput=False)
    w1h = nc.declare_dram_parameter("w1h", [P, HB, DK * P], BF16,
                                    isOutput=False)
    w2h = nc.declare_dram_parameter("w2h", [P, OB, HB * P], BF16,
                                    isOutput=False)
    b1h = nc.declare_dram_parameter("b1h", [P, HB], F32, isOutput=False)
    b2h = nc.declare_dram_parameter("b2h", [P, OB], F32, isOutput=False)
    dec = nc.declare_dram_parameter("dec", [OB, P, uc], F32, isOutput=True)

    with tile.TileContext(nc) as tc:
        with (
            tc.tile_pool(name="cpool", bufs=1) as cpool,
            tc.tile_pool(name="w1pool", bufs=3) as w1pool,
            tc.tile_pool(name="w2pool", bufs=2) as w2pool,
            tc.tile_pool(name="opool", bufs=2) as opool,
            tc.tile_pool(name="p1pool", bufs=2, space="PSUM") as p1pool,
            tc.tile_pool(name="p2pool", bufs=2, space="PSUM") as p2pool,
        ):
          for _rep in range(repeats):
            qsb = cpool.tile([P, DK, uc], BF16)
            w1tiles = [w1pool.tile([P, DK, P], BF16, name="w1sb")
                       for _ in range(2)]
            # head: first w1 chunks + q rows k-interleaved across both queues
            nc.scalar.dma_start(out=w1tiles[0][:], in_=w1h[:, 0])
            nc.sync.dma_start(out=qsb[:, 0, :], in_=qt[:, 0, :])
            nc.scalar.dma_start(out=qsb[:, 1, :], in_=qt[:, 1, :])
            nc.sync.dma_start(out=qsb[:, 2, :], in_=qt[:, 2, :])
            nc.scalar.dma_start(out=w1tiles[1][:], in_=w1h[:, 1])
            for k in range(3, DK):
                eng = nc.sync if k % 2 == 0 else nc.scalar
                eng.dma_start(out=qsb[:, k, :], in_=qt[:, k, :])
            b1sb = cpool.tile([P, HB], F32)
            nc.sync.dma_start(out=b1sb[:], in_=b1h[:])
            b2sb = cpool.tile([P, OB], F32)
            nc.sync.dma_start(out=b2sb[:], in_=b2h[:])
            hsb = cpool.tile([P, HB, uc], BF16)

            for hb in range(HB):
                if hb < 2:
                    w1sb = w1tiles[hb]
                else:
                    w1sb = w1pool.tile([P, DK, P], BF16, name="w1sb")
                    nc.scalar.dma_start(out=w1sb[:], in_=w1h[:, hb])
                psa = p1pool.tile([P, 512], F32, name="psa")
                psb = p1pool.tile([P, 512], F32, name="psb") if g2 else None
                for k in range(DK):
                    nc.tensor.matmul(psa[:, 0:g1], w1sb[:, k, :],
                                     qsb[:, k, 0:g1],
                                     start=(k == 0), stop=(k == DK - 1))
                    if g2:
                        nc.tensor.matmul(psb[:, 0:g2], w1sb[:, k, :],
                                         qsb[:, k, g1:uc],
                                         start=(k == 0), stop=(k == DK - 1))
                nc.scalar.activation(hsb[:, hb, 0:g1], psa[:, 0:g1], GELU,
                                     bias=b1sb[:, hb:hb + 1])
                if g2:
                    nc.scalar.activation(hsb[:, hb, g1:uc], psb[:, 0:g2],
                                         GELU, bias=b1sb[:, hb:hb + 1])

            for ob in range(OB):
                w2sb = w2pool.tile([P, HB, P], BF16, name="w2sb")
                nc.scalar.dma_start(out=w2sb[:], in_=w2h[:, ob])
                psa = p2pool.tile([P, 512], F32, name="psa2")
                psb = p2pool.tile([P, 512], F32, name="psb2") if g2 else None
                for k in range(HB):
                    nc.tensor.matmul(psa[:, 0:g1], w2sb[:, k, :],
                                     hsb[:, k, 0:g1],
                                     start=(k == 0), stop=(k == HB - 1))
                    if g2:
                        nc.tensor.matmul(psb[:, 0:g2], w2sb[:, k, :],
                                         hsb[:, k, g1:uc],
                                         start=(k == 0), stop=(k == HB - 1))
                osb = opool.tile([P, uc], F32, name="osb")
                nc.vector.tensor_scalar_add(osb[:, 0:g1], psa[:, 0:g1],
                                            b2sb[:, ob:ob + 1])
                nc.sync.dma_start(out=dec[ob, :, 0:g1], in_=osb[:, 0:g1])
                if g2:
                    nc.scalar.activation(osb[:, g1:uc], psb[:, 0:g2], IDENT,
                                         bias=b2sb[:, ob:ob + 1])
                    nc.sync.dma_start(out=dec[ob, :, g1:uc],
                                      in_=osb[:, g1:uc])

    nc.compile()
    return nc


def _get_nc(uc: int, repeats: int = 1):
    key = (uc, repeats)
    if key not in _cache:
        _cache[key] = _build(uc, repeats)
    return _cache[key]


def _plan(index, codebook, W1, b1, W2, b2):
    """Host-side sharding: global unique rows -> per-core transposed tiles."""
    bf = ml_dtypes.bfloat16
    flat = np.asarray(index).reshape(-1)
    uniq, inv = np.unique(flat, return_inverse=True)
    u = uniq.size
    uc = max(64, -(-u // (N_CORES * 32)) * 32)  # per-core rows, 32-aligned
    uniq_pad = np.zeros(N_CORES * uc, dtype=uniq.dtype)
    uniq_pad[:u] = uniq
    rows = np.ascontiguousarray(codebook, dtype=np.float32)[uniq_pad]
    rows = rows.astype(bf)
    # [core, t, k, p] -> [core, p, k, t]
    qt_all = np.ascontiguousarray(
        rows.reshape(N_CORES, uc, DK, P).transpose(0, 3, 2, 1))

    w1h = np.ascontiguousarray(
        W1.astype(bf).reshape(DK, P, HB, P).transpose(1, 2, 0, 3)
        .reshape(P, HB, DK * P))
    w2h = np.ascontiguousarray(
        W2.astype(bf).reshape(HB, P, OB, P).transpose(1, 2, 0, 3)
        .reshape(P, OB, HB * P))
    b1h = np.ascontiguousarray(b1.astype(np.float32).reshape(HB, P).T)
    b2h = np.ascontiguousarray(b2.astype(np.float32).reshape(OB, P).T)
    wmaps = {"w1h": w1h, "w2h": w2h, "b1h": b1h, "b2h": b2h}
    return uc, inv, qt_all, wmaps


def kernel(index, codebook, W1, b1, W2, b2):
    uc, inv, qt_all, wmaps = _plan(index, codebook, W1, b1, W2, b2)
    nc = _get_nc(uc)
    in_maps = [{**wmaps, "qt": qt_all[c]} for c in range(N_CORES)]
    res = run_bass_kernel_spmd(nc, in_maps, list(range(N_CORES)))
    # dec[ob, p, t] = rec[t, ob*128+p] -> [t, o]
    dec_all = np.concatenate(
        [res.results[c]["dec"].transpose(2, 0, 1).reshape(uc, O)
         for c in range(N_CORES)], axis=0)
    return dec_all[inv].reshape(B, M, O).astype(np.float32)


# revision 8
# speedup vs baseline: 3.1916x; 1.0647x over previous
"""VQ codebook decoder on 8 Trainium2 NeuronCores.

Strategy: the decoder output depends only on which codebook row each token
selects, so decode each *unique* referenced codebook row exactly once and
look tokens up afterwards. The host computes the global unique index set
(np.unique), pre-gathers those rows from the codebook (bf16, already in the
transposed layout mm1 wants), and splits them evenly across the 8 cores
(~U/8 rows each, padded to a multiple of 32). Each core runs the dense
2-layer MLP (1024 -> 4096 gelu -> 1024) over its rows in bf16 with fp32
PSUM accumulation and writes a decoded table; the host gathers per-token
rows from the concatenated tables during unshard (replacing the inverse
permutation scatter a per-token device output would need anyway).

Both matmuls keep weights stationary and stream tokens as the moving
operand, so tensor-engine time scales with the exact row count instead of
rounding up to 128-token tiles. Tokens are processed in two PSUM groups
(<=512 each) per weight block, issued back-to-back so each stationary
weight load is reused. Weights stream through small SBUF rings on the
Activation-engine HWDGE queue while the q rows load k-interleaved across
both queues, so the first matmul issues ~2us in. gelu+b1 ride the
scalar-engine PSUM eviction; the b2 eviction of the two final PSUM groups
runs on vector and scalar in parallel to shorten the tail.
"""

import sys

if "/opt/trn_rl_repo" not in sys.path:
    sys.path.insert(0, "/opt/trn_rl_repo")

import numpy as np
import ml_dtypes

import concourse.bass as bass
import concourse.mybir as mybir
import concourse.tile as tile
from concourse import bacc
from concourse.bass_utils import run_bass_kernel_spmd

B, M = 32, 576
CB, D, H, O = 8192, 1024, 4096, 1024
N_CORES = 8
P = 128
DK = D // P   # 8   k-subtiles for mm1
HB = H // P   # 32  column blocks of W1 / k-subtiles for mm2
OB = O // P   # 8   column blocks of W2

BF16 = mybir.dt.bfloat16
F32 = mybir.dt.float32
GELU = mybir.ActivationFunctionType.Gelu_apprx_tanh
IDENT = mybir.ActivationFunctionType.Identity

_cache: dict = {}


def _build(uc: int, repeats: int = 1, ngroups: int | None = None,
           psum_bufs: int = 2):
    """MLP over `uc` codebook rows: dec[ob*128+p, t] = rec[t, ob*128+p]."""
    if ngroups is None:
        ngroups = -(-uc // 512)
    gs = []
    rem, left = uc, ngroups
    for _ in range(ngroups):
        g = min(512, -(-rem // left // 16) * 16, rem)
        gs.append(g)
        rem -= g
        left -= 1
    bounds = [0]
    for g in gs:
        bounds.append(bounds[-1] + g)
    assert 0 < uc <= 1024 and uc % 32 == 0 and bounds[-1] == uc
    assert all(0 < g <= 512 for g in gs)

    nc = bacc.Bacc("TRN2", target_bir_lowering=False, debug=False,
                   num_devices=N_CORES)
    # qt[p, k, t] = q[t, k*128+p]; w1h[p, hb, k, c] = W1[k*128+p, hb*128+c]
    # w2h[p, ob, k, c] = W2[k*128+p, ob*128+c]; b1h/b2h column-blocked.
    qt = nc.declare_dram_parameter("qt", [P, DK, uc], BF16, isOutput=False)
    w1h = nc.declare_dram_parameter("w1h", [P, HB, DK * P], BF16,
                                    isOutput=False)
    w2h = nc.declare_dram_parameter("w2h", [P, OB, HB * P], BF16,
                                    isOutput=False)
    b1h = nc.declare_dram_parameter("b1h", [P, HB], F32, isOutput=False)
    b2h = nc.declare_dram_parameter("b2h", [P, OB], F32, isOutput=False)
    dec = nc.declare_dram_parameter("dec", [OB, P, uc], F32, isOutput=True)

    with tile.TileContext(nc) as tc:
        with (
            tc.tile_pool(name="cpool", bufs=1) as cpool,
            tc.tile_pool(name="w1pool", bufs=3) as w1pool,
            tc.tile_pool(name="w2pool", bufs=2) as w2pool,
            tc.tile_pool(name="opool", bufs=2) as opool,
            tc.tile_pool(name="p1pool", bufs=psum_bufs, space="PSUM") as p1pool,
            tc.tile_pool(name="p2pool", bufs=psum_bufs, space="PSUM") as p2pool,
        ):
          for _rep in range(repeats):
            qsb = cpool.tile([P, DK, uc], BF16)
            w1tiles = [w1pool.tile([P, DK, P], BF16, name="w1sb")
                       for _ in range(2)]
            # head: first w1 chunks + q rows k-interleaved across both queues
            nc.scalar.dma_start(out=w1tiles[0][:], in_=w1h[:, 0])
            nc.sync.dma_start(out=qsb[:, 0, :], in_=qt[:, 0, :])
            nc.scalar.dma_start(out=qsb[:, 1, :], in_=qt[:, 1, :])
            nc.sync.dma_start(out=qsb[:, 2, :], in_=qt[:, 2, :])
            nc.scalar.dma_start(out=w1tiles[1][:], in_=w1h[:, 1])
            for k in range(3, DK):
                eng = nc.sync if k % 2 == 0 else nc.scalar
                eng.dma_start(out=qsb[:, k, :], in_=qt[:, k, :])
            b1sb = cpool.tile([P, HB], F32)
            nc.sync.dma_start(out=b1sb[:], in_=b1h[:])
            b2sb = cpool.tile([P, OB], F32)
            nc.sync.dma_start(out=b2sb[:], in_=b2h[:])
            hsb = cpool.tile([P, HB, uc], BF16)

            for hb in range(HB):
                if hb < 2:
                    w1sb = w1tiles[hb]
                else:
                    w1sb = w1pool.tile([P, DK, P], BF16, name="w1sb")
                    nc.scalar.dma_start(out=w1sb[:], in_=w1h[:, hb])
                for g in range(ngroups):
                    ps = p1pool.tile([P, 512], F32, name=f"ps{g}")
                    for k in range(DK):
                        nc.tensor.matmul(ps[:, 0:gs[g]], w1sb[:, k, :],
                                         qsb[:, k, bounds[g]:bounds[g + 1]],
                                         start=(k == 0), stop=(k == DK - 1))
                    nc.scalar.activation(
                        hsb[:, hb, bounds[g]:bounds[g + 1]],
                        ps[:, 0:gs[g]], GELU, bias=b1sb[:, hb:hb + 1])

            for ob in range(OB):
                w2sb = w2pool.tile([P, HB, P], BF16, name="w2sb")
                nc.scalar.dma_start(out=w2sb[:], in_=w2h[:, ob])
                osb = opool.tile([P, uc], F32, name="osb")
                for g in range(ngroups):
                    lo, hi = bounds[g], bounds[g + 1]
                    ps = p2pool.tile([P, 512], F32, name=f"ps2{g}")
                    for k in range(HB):
                        nc.tensor.matmul(ps[:, 0:gs[g]], w2sb[:, k, :],
                                         hsb[:, k, lo:hi],
                                         start=(k == 0), stop=(k == HB - 1))
                    if g % 2 == 0:
                        nc.vector.tensor_scalar_add(osb[:, lo:hi],
                                                    ps[:, 0:gs[g]],
                                                    b2sb[:, ob:ob + 1])
                    else:
                        nc.scalar.activation(osb[:, lo:hi], ps[:, 0:gs[g]],
                                             IDENT, bias=b2sb[:, ob:ob + 1])
                    nc.sync.dma_start(out=dec[ob, :, lo:hi], in_=osb[:, lo:hi])

    nc.compile()
    return nc


def _get_nc(uc: int, repeats: int = 1, ngroups: int | None = None,
            psum_bufs: int = 2):
    key = (uc, repeats, ngroups, psum_bufs)
    if key not in _cache:
        _cache[key] = _build(uc, repeats, ngroups, psum_bufs)
    return _cache[key]


def _plan(index, codebook, W1, b1, W2, b2):
    """Host-side sharding: global unique rows -> per-core transposed tiles."""
    bf = ml_dtypes.bfloat16
    flat = np.asarray(index).reshape(-1)
    uniq, inv = np.unique(flat, return_inverse=True)
    u = uniq.size
    uc = max(64, -(-u // (N_CORES * 32)) * 32)  # per-core rows, 32-aligned
    uniq_pad = np.zeros(N_CORES * uc, dtype=uniq.dtype)
    uniq_pad[:u] = uniq
    rows = np.ascontiguousarray(codebook, dtype=np.float32)[uniq_pad]
    rows = rows.astype(bf)
    # [core, t, k, p] -> [core, p, k, t]
    qt_all = np.ascontiguousarray(
        rows.reshape(N_CORES, uc, DK, P).transpose(0, 3, 2, 1))

    w1h = np.ascontiguousarray(
        W1.astype(bf).reshape(DK, P, HB, P).transpose(1, 2, 0, 3)
        .reshape(P, HB, DK * P))
    w2h = np.ascontiguousarray(
        W2.astype(bf).reshape(HB, P, OB, P).transpose(1, 2, 0, 3)
        .reshape(P, OB, HB * P))
    b1h = np.ascontiguousarray(b1.astype(np.float32).reshape(HB, P).T)
    b2h = np.ascontiguousarray(b2.astype(np.float32).reshape(OB, P).T)
    wmaps = {"w1h": w1h, "w2h": w2h, "b1h": b1h, "b2h": b2h}
    return uc, inv, qt_all, wmaps


def kernel(index, codebook, W1, b1, W2, b2):
    uc, inv, qt_all, wmaps = _plan(index, codebook, W1, b1, W2, b2)
    nc = _get_nc(uc)
    in_maps = [{**wmaps, "qt": qt_all[c]} for c in range(N_CORES)]
    res = run_bass_kernel_spmd(nc, in_maps, list(range(N_CORES)))
    # dec[ob, p, t] = rec[t, ob*128+p] -> [t, o]
    dec_all = np.concatenate(
        [res.results[c]["dec"].transpose(2, 0, 1).reshape(uc, O)
         for c in range(N_CORES)], axis=0)
    return dec_all[inv].reshape(B, M, O).astype(np.float32)
